# revision 4
# baseline (speedup 1.0000x reference)
"""GraphSAGE (2-layer SAGEConv + log_softmax) on 8 Trainium2 NeuronCores.

Sharding: nodes partitioned contiguously across 8 cores (6250 each, padded
to 6400 = 50 tiles of 128 slots). Within a core, nodes are dealt to tiles
round-robin by in-degree so per-tile edge counts are balanced.

Math restructure (exact up to fp reassociation):
  l1: mean = segsum_e(w_e * x[src_e]) with w_e = 1/max(deg(dst_e),1)
      h = relu(mean @ Wl1 + b1 + x @ Wr1)
  l2: z = h @ Wl2 ; r = h @ Wr2          (applied before aggregation -
      valid since segment-mean commutes with the linear map)
      out = log_softmax(segmean_e(z[src_e]) + b2 + r)

Phase 1 kernel (per core): gather x rows per edge chunk (indirect DMA),
one-hot weighted segment-sum via TensorE, h^T = relu(Wl1^T magg + Wr1^T x^T
+ b1) in f32r, z|r = (Wl2|Wr2)^T h via f32r, PE-transpose back -> z, r.
Host: concat z shards -> z_all. Phase 2 kernel: gather z rows per edge,
segment-sum, + r + b2, row log_softmax.
"""
import math
import numpy as np
import ml_dtypes

import concourse.bass as bass
import concourse.bacc as bacc
import concourse.mybir as mybir
import concourse.tile as tile
from concourse import bass_utils

F32 = mybir.dt.float32
F32R = mybir.dt.float32r
BF16 = mybir.dt.bfloat16
I32 = mybir.dt.int32
AF = mybir.ActivationFunctionType
OP = mybir.AluOpType
P = 128

# problem constants (hardcoded per contract)
N_NODES = 50000
N_EDGES = 400000
IN_CH = 128
HID = 1024
OUT_CH = 47
NCORES = 8
NPC = N_NODES // NCORES          # nodes per core (6250)
NTILES = 50                      # padded tiles per core
SLOTS = NTILES * P               # 6400 padded slots per core
OUTP = 64                        # padded z/r/out row width
HB = HID // P                    # 8 hid blocks


def build_phase1(chunks: int):
    nc = bacc.Bacc("TRN2", target_bir_lowering=False, debug=False,
                   enable_asserts=False, num_devices=NCORES)
    x_full = nc.dram_tensor("x_full", [N_NODES, IN_CH], F32, kind="ExternalInput").ap()
    x_shard = nc.dram_tensor("x_shard", [SLOTS, IN_CH], F32, kind="ExternalInput").ap()
    idx1 = nc.dram_tensor("idx1", [P, NTILES, chunks], I32, kind="ExternalInput").ap()
    dstv = nc.dram_tensor("dstv", [P, NTILES, chunks], BF16, kind="ExternalInput").ap()
    win = nc.dram_tensor("win", [P, NTILES, chunks], F32, kind="ExternalInput").ap()
    Wl1 = nc.dram_tensor("Wl1", [IN_CH, HID], F32, kind="ExternalInput").ap()
    Wr1 = nc.dram_tensor("Wr1", [IN_CH, HID], F32, kind="ExternalInput").ap()
    WLR2 = nc.dram_tensor("WLR2", [HID, 2 * OUT_CH], F32, kind="ExternalInput").ap()
    b1c = nc.dram_tensor("b1c", [P, HB], F32, kind="ExternalInput").ap()
    iota = nc.dram_tensor("iota", [P, P], BF16, kind="ExternalInput").ap()
    ident = nc.dram_tensor("ident", [P, P], F32, kind="ExternalInput").ap()

    z_out = nc.dram_tensor("z_out", [SLOTS, OUTP], F32, kind="ExternalOutput").ap()
    r_out = nc.dram_tensor("r_out", [SLOTS, OUTP], F32, kind="ExternalOutput").ap()

    with tile.TileContext(nc) as tc:
        with (
            tc.tile_pool(name="const", bufs=1) as cp,
            tc.tile_pool(name="work", bufs=3) as wp,
            tc.tile_pool(name="stage", bufs=2) as sp,
            tc.tile_pool(name="ps_mag", bufs=2, space="PSUM") as psm,
            tc.tile_pool(name="ps_one", bufs=1, space="PSUM") as pss,
            tc.tile_pool(name="ps_h", bufs=3, space="PSUM") as psh,
        ):
            # ---- constants / weights ----
            idx_sb = cp.tile([P, NTILES, chunks], I32)
            nc.sync.dma_start(out=idx_sb[:], in_=idx1)
            dstv_sb = cp.tile([P, NTILES, chunks], BF16)
            nc.sync.dma_start(out=dstv_sb[:], in_=dstv)
            w_sb = cp.tile([P, NTILES, chunks], F32)
            nc.sync.dma_start(out=w_sb[:], in_=win)
            iota_sb = cp.tile([P, P], BF16)
            nc.sync.dma_start(out=iota_sb[:], in_=iota)
            id_sb = cp.tile([P, P], F32)
            nc.sync.dma_start(out=id_sb[:], in_=ident)
            b1_sb = cp.tile([P, HB], F32)
            nc.sync.dma_start(out=b1_sb[:], in_=b1c)

            wl1_f = cp.tile([P, HID], F32)
            nc.sync.dma_start(out=wl1_f[:], in_=Wl1)
            wl1_r = cp.tile([P, HID], F32R)
            nc.vector.tensor_copy(out=wl1_r[:], in_=wl1_f[:])
            wr1_f = cp.tile([P, HID], F32)
            nc.sync.dma_start(out=wr1_f[:], in_=Wr1)
            wr1_r = cp.tile([P, HID], F32R)
            nc.vector.tensor_copy(out=wr1_r[:], in_=wr1_f[:])
            w2_f = cp.tile([P, HB, 2 * OUT_CH], F32)
            nc.sync.dma_start(
                out=w2_f[:], in_=WLR2.rearrange("(j p) o -> p j o", p=P))
            w2_r = cp.tile([P, HB, 2 * OUT_CH], F32R)
            nc.vector.tensor_copy(out=w2_r[:], in_=w2_f[:])

            for pair in range(NTILES // 2):
                mag_r = wp.tile([P, 2 * P], F32R, tag="mag")
                xt_r = wp.tile([P, 2 * P], F32R, tag="xt")
                for half in range(2):
                    t = 2 * pair + half
                    # gather x rows for this tile's edge chunks
                    m_f = wp.tile([P, chunks, IN_CH], F32, tag="m")
                    for c in range(chunks):
                        nc.gpsimd.indirect_dma_start(
                            out=m_f[:, c, :], out_offset=None, in_=x_full,
                            in_offset=bass.IndirectOffsetOnAxis(
                                ap=idx_sb[:, t, c:c + 1], axis=0))
                    # cast + per-edge weight scale -> bf16
                    m_b = wp.tile([P, chunks, IN_CH], BF16, tag="mb")
                    nc.vector.tensor_tensor(
                        out=m_b[:], in0=m_f[:],
                        in1=w_sb[:, t, :].to_broadcast([P, chunks, IN_CH]),
                        op=OP.mult)
                    # one-hot [e, chunks, dst128]
                    oh = wp.tile([P, chunks, P], BF16, tag="oh")
                    nc.vector.tensor_tensor(
                        out=oh[:],
                        in0=dstv_sb[:, t, :].to_broadcast([P, chunks, P]),
                        in1=iota_sb[:].rearrange("p (c d) -> p c d", c=1).to_broadcast([P, chunks, P]),
                        op=OP.is_equal)
                    # weighted segment sum: magT[ch, dst] += M^T OH
                    ps_mag = psm.tile([P, P], F32, space="PSUM", tag="psmag")
                    for c in range(chunks):
                        nc.tensor.matmul(
                            out=ps_mag[:], lhsT=m_b[:, c, :], rhs=oh[:, c, :],
                            start=(c == 0), stop=(c == chunks - 1))
                    nc.vector.tensor_copy(out=mag_r[:, half * P:(half + 1) * P], in_=ps_mag[:])
                    # x tile load + PE transpose
                    xt_f = wp.tile([P, P], F32, tag="xtf")
                    nc.sync.dma_start(out=xt_f[:], in_=x_shard[t * P:(t + 1) * P, :])
                    ps_xt = pss.tile([P, P], F32, space="PSUM", tag="psxt")
                    nc.tensor.transpose(out=ps_xt[:], in_=xt_f[:], identity=id_sb[:])
                    nc.vector.tensor_copy(out=xt_r[:, half * P:(half + 1) * P], in_=ps_xt[:])

                # hT blocks: [hid128, 256] = Wl1_j^T magg + Wr1_j^T xT
                ht_r = sp.tile([P, HB, 2 * P], F32R, tag="ht")
                for j in range(HB):
                    ps_ht = psh.tile([P, 2 * P], F32, space="PSUM", tag="psht")
                    nc.tensor.matmul(out=ps_ht[:], lhsT=wl1_r[:, j * P:(j + 1) * P],
                                     rhs=mag_r[:], start=True, stop=False)
                    nc.tensor.matmul(out=ps_ht[:], lhsT=wr1_r[:, j * P:(j + 1) * P],
                                     rhs=xt_r[:], start=False, stop=True)
                    # relu(x + b1_j), round to f32r
                    nc.scalar.activation(out=ht_r[:, j, :], in_=ps_ht[:],
                                         func=AF.Relu, bias=b1_sb[:, j:j + 1], scale=1.0)
                # zrT [94, 256] = (Wl2|Wr2)^T h
                ps_zr = pss.tile([2 * OUT_CH, 2 * P], F32, space="PSUM", tag="pszr")
                for j in range(HB):
                    nc.tensor.matmul(out=ps_zr[:], lhsT=w2_r[:, j, :], rhs=ht_r[:, j, :],
                                     start=(j == 0), stop=(j == HB - 1))
                zr_f = sp.tile([2 * OUT_CH, 2 * P], F32, tag="zrf")
                nc.vector.tensor_copy(out=zr_f[:], in_=ps_zr[:])
                # transpose back per 128-node half: [94,128] -> [128,94]
                zst = sp.tile([P, 2, OUTP], F32, tag="zst")
                nc.vector.memset(zst[:], 0.0)
                rst = sp.tile([P, 2, OUTP], F32, tag="rst")
                for half in range(2):
                    ps_t = pss.tile([P, 2 * OUT_CH], F32, space="PSUM", tag="pst")
                    nc.tensor.transpose(out=ps_t[:], in_=zr_f[:, half * P:(half + 1) * P],
                                        identity=id_sb[0:2 * OUT_CH, 0:2 * OUT_CH])
                    nc.vector.tensor_copy(out=zst[:, half, 0:OUT_CH], in_=ps_t[:, 0:OUT_CH])
                    nc.vector.tensor_copy(out=rst[:, half, 0:OUT_CH],
                                          in_=ps_t[:, OUT_CH:2 * OUT_CH])
                nc.sync.dma_start(
                    out=z_out[pair * 2 * P:(pair + 1) * 2 * P, :].rearrange(
                        "(t p) c -> p t c", p=P),
                    in_=zst[:])
                nc.sync.dma_start(
                    out=r_out[pair * 2 * P:(pair + 1) * 2 * P, :].rearrange(
                        "(t p) c -> p t c", p=P),
                    in_=rst[:])
    nc.compile()
    return nc


def build_phase2(chunks: int):
    nc = bacc.Bacc("TRN2", target_bir_lowering=False, debug=False,
                   enable_asserts=False, num_devices=NCORES)
    z_all = nc.dram_tensor("z_all", [NCORES * SLOTS, OUTP], F32, kind="ExternalInput").ap()
    idx2 = nc.dram_tensor("idx2", [P, NTILES, chunks], I32, kind="ExternalInput").ap()
    dstv = nc.dram_tensor("dstv", [P, NTILES, chunks], BF16, kind="ExternalInput").ap()
    win = nc.dram_tensor("win", [P, NTILES, chunks], F32, kind="ExternalInput").ap()
    r_in = nc.dram_tensor("r_in", [SLOTS, OUTP], F32, kind="ExternalInput").ap()
    b2rep = nc.dram_tensor("b2rep", [P, OUTP], F32, kind="ExternalInput").ap()
    iota = nc.dram_tensor("iota", [P, P], BF16, kind="ExternalInput").ap()
    out = nc.dram_tensor("out", [SLOTS, OUT_CH], F32, kind="ExternalOutput").ap()

    with tile.TileContext(nc) as tc:
        with (
            tc.tile_pool(name="const", bufs=1) as cp,
            tc.tile_pool(name="work", bufs=3) as wp,
            tc.tile_pool(name="ps", bufs=4, space="PSUM") as ps,
        ):
            idx_sb = cp.tile([P, NTILES, chunks], I32)
            nc.sync.dma_start(out=idx_sb[:], in_=idx2)
            dstv_sb = cp.tile([P, NTILES, chunks], BF16)
            nc.sync.dma_start(out=dstv_sb[:], in_=dstv)
            w_sb = cp.tile([P, NTILES, chunks], F32)
            nc.sync.dma_start(out=w_sb[:], in_=win)
            iota_sb = cp.tile([P, P], BF16)
            nc.sync.dma_start(out=iota_sb[:], in_=iota)
            b2_sb = cp.tile([P, OUTP], F32)
            nc.sync.dma_start(out=b2_sb[:], in_=b2rep)
            out_stage = cp.tile([P, NTILES, OUT_CH], F32)

            for t in range(NTILES):
                m_f = wp.tile([P, chunks, OUTP], F32, tag="m")
                for c in range(chunks):
                    nc.gpsimd.indirect_dma_start(
                        out=m_f[:, c, :], out_offset=None, in_=z_all,
                        in_offset=bass.IndirectOffsetOnAxis(
                            ap=idx_sb[:, t, c:c + 1], axis=0))
                m_b = wp.tile([P, chunks, OUTP], BF16, tag="mb")
                nc.vector.tensor_tensor(
                    out=m_b[:], in0=m_f[:],
                    in1=w_sb[:, t, :].to_broadcast([P, chunks, OUTP]),
                    op=OP.mult)
                oh = wp.tile([P, chunks, P], BF16, tag="oh")
                nc.vector.tensor_tensor(
                    out=oh[:],
                    in0=dstv_sb[:, t, :].to_broadcast([P, chunks, P]),
                    in1=iota_sb[:].rearrange("p (c d) -> p c d", c=1).to_broadcast([P, chunks, P]),
                    op=OP.is_equal)
                ps_o = ps.tile([P, OUTP], F32, space="PSUM", tag="pso")
                for c in range(chunks):
                    nc.tensor.matmul(out=ps_o[:], lhsT=oh[:, c, :], rhs=m_b[:, c, :],
                                     start=(c == 0), stop=(c == chunks - 1))
                # t = agg + r + b2
                r_t = wp.tile([P, OUTP], F32, tag="rt")
                nc.sync.dma_start(out=r_t[:], in_=r_in[t * P:(t + 1) * P, :])
                t_sb = wp.tile([P, OUTP], F32, tag="tsb")
                nc.vector.tensor_tensor(out=t_sb[:], in0=ps_o[:], in1=r_t[:], op=OP.add)
                nc.vector.tensor_tensor(out=t_sb[:], in0=t_sb[:], in1=b2_sb[:], op=OP.add)
                # log_softmax over first OUT_CH cols
                rmax = wp.tile([P, 1], F32, tag="rmax")
                nc.vector.tensor_reduce(out=rmax[:], in_=t_sb[:, 0:OUT_CH],
                                        axis=mybir.AxisListType.X, op=OP.max)
                nmax = wp.tile([P, 1], F32, tag="nmax")
                nc.vector.tensor_scalar_mul(out=nmax[:], in0=rmax[:], scalar1=-1.0)
                e_sb = wp.tile([P, OUT_CH], F32, tag="esb")
                sume = wp.tile([P, 1], F32, tag="sume")
                nc.scalar.activation(out=e_sb[:], in_=t_sb[:, 0:OUT_CH], func=AF.Exp,
                                     bias=nmax[:], scale=1.0, accum_out=sume[:])
                lse = wp.tile([P, 1], F32, tag="lse")
                nc.scalar.activation(out=lse[:], in_=sume[:], func=AF.Ln)
                nc.vector.tensor_scalar(
                    out=out_stage[:, t, :], in0=t_sb[:, 0:OUT_CH],
                    scalar1=nmax[:], scalar2=lse[:], op0=OP.add, op1=OP.subtract)
            nc.sync.dma_start(
                out=out.rearrange("(t p) c -> p t c", p=P), in_=out_stage[:])
    nc.compile()
    return nc


def _prep(x, edge_index, Wl1, Wr1, b1, Wl2, Wr2, b2):
    """Host-side integer/layout preprocessing. Returns per-core input maps
    and the unpermutation for assembling the final output."""
    src = edge_index[0].astype(np.int64)
    dst = edge_index[1].astype(np.int64)
    deg = np.bincount(dst, minlength=N_NODES)
    winv = np.zeros(N_NODES, np.float32)
    winv = 1.0 / np.maximum(deg, 1).astype(np.float32)

    # per-core slot assignment: round-robin by degree over tiles
    slot_of = np.empty(N_NODES, np.int64)
    for c in range(NCORES):
        nids = np.arange(c * NPC, (c + 1) * NPC)
        order = nids[np.argsort(-deg[nids], kind="stable")]
        slots = np.empty(NPC, np.int64)
        # deal: node k -> tile k%NTILES, position k//NTILES
        k = np.arange(NPC)
        slots = (k % NTILES) * P + (k // NTILES)
        slot_of[order] = slots  # within-core slot
    core_of = np.minimum(dst // NPC, NCORES - 1)

    # group edges by (core, tile)
    dslot = slot_of[dst]
    dtile = dslot // P
    dlane = dslot % P
    ecore = dst // NPC

    max_chunk = 0
    per_core = []
    for c in range(NCORES):
        sel = np.nonzero(ecore == c)[0]
        # sort by (tile, src) for locality
        o = np.lexsort((src[sel], dtile[sel]))
        sel = sel[o]
        t_arr = dtile[sel]
        counts = np.bincount(t_arr, minlength=NTILES)
        max_chunk = max(max_chunk, int(np.ceil(counts.max() / P)))
        per_core.append((sel, counts))

    chunks = max(int(max_chunk), 1)

    idx1 = np.zeros((NCORES, P, NTILES, chunks), np.int32)
    idx2 = np.zeros((NCORES, P, NTILES, chunks), np.int32)
    dstv = np.full((NCORES, P, NTILES, chunks), -1.0, np.float32)
    wvals = np.zeros((NCORES, P, NTILES, chunks), np.float32)
    for c in range(NCORES):
        sel, counts = per_core[c]
        off = 0
        for t in range(NTILES):
            cnt = counts[t]
            es = sel[off:off + cnt]
            off += cnt
            k = np.arange(cnt)
            ch = k // P
            lane = k % P
            idx1[c, lane, t, ch] = src[es]
            sc = np.minimum(src[es] // NPC, NCORES - 1)
            idx2[c, lane, t, ch] = sc * SLOTS + slot_of[src[es]]
            dstv[c, lane, t, ch] = dlane[es]
            wvals[c, lane, t, ch] = winv[dst[es]]

    iota = np.tile(np.arange(P, dtype=np.float32)[None, :], (P, 1))
    ident = np.eye(P, dtype=np.float32)
    b1c = b1.reshape(HB, P).T.astype(np.float32).copy()  # [128, HB]
    WLR2 = np.concatenate([Wl2, Wr2], axis=1).astype(np.float32)  # [HID, 94]
    b2rep = np.zeros((P, OUTP), np.float32)
    b2rep[:, :OUT_CH] = b2[None, :]

    in1_maps, in2_maps = [], []
    xs_pad = np.zeros((NCORES, SLOTS, IN_CH), np.float32)
    for c in range(NCORES):
        nids = np.arange(c * NPC, (c + 1) * NPC)
        xs_pad[c, slot_of[nids], :] = x[nids]
        in1_maps.append({
            "x_full": x, "x_shard": xs_pad[c],
            "idx1": idx1[c], "dstv": dstv[c].astype(ml_dtypes.bfloat16),
            "win": wvals[c],
            "Wl1": Wl1, "Wr1": Wr1, "WLR2": WLR2, "b1c": b1c,
            "iota": iota.astype(ml_dtypes.bfloat16), "ident": ident,
        })
        in2_maps.append({
            "idx2": idx2[c], "dstv": dstv[c].astype(ml_dtypes.bfloat16),
            "win": wvals[c], "b2rep": b2rep,
            "iota": iota.astype(ml_dtypes.bfloat16),
        })
    return chunks, in1_maps, in2_maps, slot_of


_cache = {}


def kernel(x, edge_index, Wl1, Wr1, b1, Wl2, Wr2, b2):
    x = np.asarray(x, np.float32)
    edge_index = np.asarray(edge_index)
    chunks, in1_maps, in2_maps, slot_of = _prep(
        x, edge_index, np.asarray(Wl1, np.float32), np.asarray(Wr1, np.float32),
        np.asarray(b1, np.float32), np.asarray(Wl2, np.float32),
        np.asarray(Wr2, np.float32), np.asarray(b2, np.float32))

    if ("p1", chunks) not in _cache:
        _cache[("p1", chunks)] = build_phase1(chunks)
    nc1 = _cache[("p1", chunks)]
    res1 = bass_utils.run_bass_kernel_spmd(nc1, in1_maps, core_ids=list(range(NCORES)))
    z_all = np.concatenate([res1.results[c]["z_out"] for c in range(NCORES)], axis=0)
    for c in range(NCORES):
        in2_maps[c]["z_all"] = z_all
        in2_maps[c]["r_in"] = res1.results[c]["r_out"]

    if ("p2", chunks) not in _cache:
        _cache[("p2", chunks)] = build_phase2(chunks)
    nc2 = _cache[("p2", chunks)]
    res2 = bass_utils.run_bass_kernel_spmd(nc2, in2_maps, core_ids=list(range(NCORES)))

    out = np.empty((N_NODES, OUT_CH), np.float32)
    for c in range(NCORES):
        o = res2.results[c]["out"]  # [SLOTS, OUT_CH]
        nids = np.arange(c * NPC, (c + 1) * NPC)
        out[nids] = o[slot_of[nids]]
    return out


# ---------------------------------------------------------------------------
# timing utilities (axon has no NTFF profiling; estimate device time by
# repeated execution on persistent device buffers minus an empty baseline)
# ---------------------------------------------------------------------------

def _make_runner(nc, n_cores):
    import time
    import jax
    from jax.sharding import Mesh, PartitionSpec, NamedSharding
    from jax.experimental.shard_map import shard_map
    from concourse import bass2jax

    bass2jax.install_neuronx_cc_hook()
    in_names, out_names, out_avals = [], [], []
    for alloc in nc.m.functions[0].allocations:
        if not isinstance(alloc, mybir.MemoryLocationSet):
            continue
        name = alloc.memorylocations[0].name
        if alloc.kind == "ExternalInput":
            in_names.append(name)
        elif alloc.kind == "ExternalOutput":
            out_names.append(name)
            out_avals.append(jax.core.ShapedArray(
                tuple(alloc.tensor_shape), mybir.dt.np(alloc.dtype)))
    n_params = len(in_names)
    all_in = list(in_names) + list(out_names)

    def _body(*args):
        outs = bass2jax._bass_exec_p.bind(
            *args, out_avals=tuple(out_avals), in_names=tuple(all_in),
            out_names=tuple(out_names), lowering_input_output_aliases=(),
            sim_require_finite=False, sim_require_nnan=False, nc=nc)
        return tuple(outs)

    devices = jax.devices()[:n_cores]
    mesh = Mesh(np.asarray(devices), ("core",))
    jitted = jax.jit(
        shard_map(_body, mesh=mesh,
                  in_specs=(PartitionSpec("core"),) * (n_params + len(out_names)),
                  out_specs=(PartitionSpec("core"),) * len(out_names),
                  check_rep=False),
        keep_unused=True)

    def prep(in_maps):
        concat = [np.concatenate([np.asarray(in_maps[c][n]) for c in range(n_cores)], 0)
                  for n in in_names]
        zeros = [np.zeros((n_cores * a.shape[0], *a.shape[1:]), a.dtype)
                 for a in out_avals]
        sh = NamedSharding(mesh, PartitionSpec("core"))
        return [jax.device_put(v, sh) for v in concat + zeros]

    def timed(dev_in, iters):
        import jax
        out = jitted(*dev_in)
        jax.block_until_ready(out)
        t0 = time.perf_counter()
        for _ in range(iters):
            out = jitted(*dev_in)
            jax.block_until_ready(out)
        return out, (time.perf_counter() - t0) / iters

    return prep, timed, out_names


def _build_empty():
    nc = bacc.Bacc("TRN2", target_bir_lowering=False, debug=False,
                   enable_asserts=False, num_devices=NCORES)
    a = nc.dram_tensor("a", [P, P], F32, kind="ExternalInput").ap()
    o = nc.dram_tensor("o", [P, P], F32, kind="ExternalOutput").ap()
    with tile.TileContext(nc) as tc:
        with tc.tile_pool(name="sb", bufs=1) as sb:
            t = sb.tile([P, P], F32)
            nc.sync.dma_start(out=t[:], in_=a)
            nc.sync.dma_start(out=o, in_=t[:])
    nc.compile()
    return nc


def measure_exec_ns(inp, iters=8):
    """Differential device-time estimate for the two kernel phases."""
    import jax
    chunks, in1_maps, in2_maps, slot_of = _prep(
        np.asarray(inp["x"], np.float32), np.asarray(inp["edge_index"]),
        np.asarray(inp["Wl1"], np.float32), np.asarray(inp["Wr1"], np.float32),
        np.asarray(inp["b1"], np.float32), np.asarray(inp["Wl2"], np.float32),
        np.asarray(inp["Wr2"], np.float32), np.asarray(inp["b2"], np.float32))
    if ("p1", chunks) not in _cache:
        _cache[("p1", chunks)] = build_phase1(chunks)
    if ("p2", chunks) not in _cache:
        _cache[("p2", chunks)] = build_phase2(chunks)

    prep0, timed0, _ = _make_runner(_build_empty(), NCORES)
    d0 = prep0([{"a": np.zeros((P, P), np.float32)} for _ in range(NCORES)])
    _, t_base = timed0(d0, iters)

    prep1, timed1, names1 = _make_runner(_cache[("p1", chunks)], NCORES)
    d1 = prep1(in1_maps)
    out1, t1 = timed1(d1, iters)
    zi = names1.index("z_out")
    ri = names1.index("r_out")
    z_all = np.asarray(out1[zi]).reshape(NCORES * SLOTS, OUTP)
    for c in range(NCORES):
        in2_maps[c]["z_all"] = z_all
        in2_maps[c]["r_in"] = np.asarray(out1[ri]).reshape(NCORES, SLOTS, OUTP)[c]

    prep2, timed2, _ = _make_runner(_cache[("p2", chunks)], NCORES)
    d2 = prep2(in2_maps)
    _, t2 = timed2(d2, iters)

    est = max(t1 - t_base, 0.0) + max(t2 - t_base, 0.0)
    print(f"  [timing] base {t_base*1e3:.2f} ms, p1 {t1*1e3:.2f} ms, "
          f"p2 {t2*1e3:.2f} ms -> est {est*1e3:.2f} ms")
    return int(est * 1e9)


# revision 6
# speedup vs baseline: 6328.3302x; 6328.3302x over previous
"""GraphSAGE (2-layer SAGEConv + log_softmax) on 8 Trainium2 NeuronCores.

Sharding: nodes partitioned contiguously across 8 cores (6250 each, padded
to 6400 = 50 tiles of 128 slots). Within a core, nodes are dealt to tiles
round-robin by in-degree so per-tile edge counts are balanced.

Math restructure (exact up to fp reassociation):
  l1: mean = segsum_e(w_e * x[src_e]) with w_e = 1/max(deg(dst_e),1)
      h = relu(mean @ Wl1 + b1 + x @ Wr1)
  l2: z = h @ Wl2 ; r = h @ Wr2          (applied before aggregation -
      valid since segment-mean commutes with the linear map)
      out = log_softmax(segmean_e(z[src_e]) + b2 + r)

Phase 1 kernel (per core): gather x rows per edge chunk (indirect DMA),
one-hot weighted segment-sum via TensorE, h^T = relu(Wl1^T magg + Wr1^T x^T
+ b1) in f32r, z|r = (Wl2|Wr2)^T h via f32r, PE-transpose back -> z, r.
Host: concat z shards -> z_all. Phase 2 kernel: gather z rows per edge,
segment-sum, + r + b2, row log_softmax.
"""
import math
import numpy as np
import ml_dtypes

import concourse.bass as bass
import concourse.bacc as bacc
import concourse.mybir as mybir
import concourse.tile as tile
from concourse import bass_utils

F32 = mybir.dt.float32
F32R = mybir.dt.float32r
BF16 = mybir.dt.bfloat16
I32 = mybir.dt.int32
AF = mybir.ActivationFunctionType
OP = mybir.AluOpType
P = 128

# problem constants (hardcoded per contract)
N_NODES = 50000
N_EDGES = 400000
IN_CH = 128
HID = 1024
OUT_CH = 47
NCORES = 8
NPC = N_NODES // NCORES          # nodes per core (6250)
NTILES = 50                      # padded tiles per core
SLOTS = NTILES * P               # 6400 padded slots per core
OUTP = 64                        # padded z/r/out row width
HB = HID // P                    # 8 hid blocks


def build_phase1(chunks: int):
    nc = bacc.Bacc("TRN2", target_bir_lowering=False, debug=False,
                   enable_asserts=False, num_devices=NCORES)
    x_full = nc.dram_tensor("x_full", [N_NODES, IN_CH], F32, kind="ExternalInput").ap()
    x_shard = nc.dram_tensor("x_shard", [SLOTS, IN_CH], F32, kind="ExternalInput").ap()
    idx1 = nc.dram_tensor("idx1", [P, NTILES, chunks], I32, kind="ExternalInput").ap()
    dstv = nc.dram_tensor("dstv", [P, NTILES, chunks], BF16, kind="ExternalInput").ap()
    win = nc.dram_tensor("win", [P, NTILES, chunks], F32, kind="ExternalInput").ap()
    Wl1 = nc.dram_tensor("Wl1", [IN_CH, HID], F32, kind="ExternalInput").ap()
    Wr1 = nc.dram_tensor("Wr1", [IN_CH, HID], F32, kind="ExternalInput").ap()
    WLR2 = nc.dram_tensor("WLR2", [HID, 2 * OUT_CH], F32, kind="ExternalInput").ap()
    b1c = nc.dram_tensor("b1c", [P, HB], F32, kind="ExternalInput").ap()
    iota = nc.dram_tensor("iota", [P, P], BF16, kind="ExternalInput").ap()
    ident = nc.dram_tensor("ident", [P, P], F32, kind="ExternalInput").ap()

    z_out = nc.dram_tensor("z_out", [SLOTS, OUTP], F32, kind="ExternalOutput").ap()
    r_out = nc.dram_tensor("r_out", [SLOTS, OUTP], F32, kind="ExternalOutput").ap()

    with tile.TileContext(nc) as tc:
        with (
            tc.tile_pool(name="const", bufs=1) as cp,
            tc.tile_pool(name="work", bufs=3) as wp,
            tc.tile_pool(name="stage", bufs=2) as sp,
            tc.tile_pool(name="ps_mag", bufs=2, space="PSUM") as psm,
            tc.tile_pool(name="ps_one", bufs=1, space="PSUM") as pss,
            tc.tile_pool(name="ps_h", bufs=3, space="PSUM") as psh,
        ):
            # ---- constants / weights ----
            idx_sb = cp.tile([P, NTILES, chunks], I32)
            nc.sync.dma_start(out=idx_sb[:], in_=idx1)
            dstv_sb = cp.tile([P, NTILES, chunks], BF16)
            nc.sync.dma_start(out=dstv_sb[:], in_=dstv)
            w_sb = cp.tile([P, NTILES, chunks], F32)
            nc.sync.dma_start(out=w_sb[:], in_=win)
            iota_sb = cp.tile([P, P], BF16)
            nc.sync.dma_start(out=iota_sb[:], in_=iota)
            id_sb = cp.tile([P, P], F32)
            nc.sync.dma_start(out=id_sb[:], in_=ident)
            b1_sb = cp.tile([P, HB], F32)
            nc.sync.dma_start(out=b1_sb[:], in_=b1c)

            wl1_f = cp.tile([P, HID], F32)
            nc.sync.dma_start(out=wl1_f[:], in_=Wl1)
            wl1_r = cp.tile([P, HID], F32R)
            nc.vector.tensor_copy(out=wl1_r[:], in_=wl1_f[:])
            wr1_f = cp.tile([P, HID], F32)
            nc.sync.dma_start(out=wr1_f[:], in_=Wr1)
            wr1_r = cp.tile([P, HID], F32R)
            nc.vector.tensor_copy(out=wr1_r[:], in_=wr1_f[:])
            w2_f = cp.tile([P, HB, 2 * OUT_CH], F32)
            nc.sync.dma_start(
                out=w2_f[:], in_=WLR2.rearrange("(j p) o -> p j o", p=P))
            w2_r = cp.tile([P, HB, 2 * OUT_CH], F32R)
            nc.vector.tensor_copy(out=w2_r[:], in_=w2_f[:])

            for pair in range(NTILES // 2):
                mag_r = wp.tile([P, 2 * P], F32R, tag="mag")
                xt_r = wp.tile([P, 2 * P], F32R, tag="xt")
                for half in range(2):
                    t = 2 * pair + half
                    # gather x rows for this tile's edge chunks
                    m_f = wp.tile([P, chunks, IN_CH], F32, tag="m")
                    for c in range(chunks):
                        nc.gpsimd.indirect_dma_start(
                            out=m_f[:, c, :], out_offset=None, in_=x_full,
                            in_offset=bass.IndirectOffsetOnAxis(
                                ap=idx_sb[:, t, c:c + 1], axis=0))
                    # cast + per-edge weight scale -> bf16
                    m_b = wp.tile([P, chunks, IN_CH], BF16, tag="mb")
                    nc.vector.tensor_tensor(
                        out=m_b[:], in0=m_f[:],
                        in1=w_sb[:, t, :].to_broadcast([P, chunks, IN_CH]),
                        op=OP.mult)
                    # one-hot [e, chunks, dst128]
                    oh = wp.tile([P, chunks, P], BF16, tag="oh")
                    nc.vector.tensor_tensor(
                        out=oh[:],
                        in0=dstv_sb[:, t, :].to_broadcast([P, chunks, P]),
                        in1=iota_sb[:].rearrange("p (c d) -> p c d", c=1).to_broadcast([P, chunks, P]),
                        op=OP.is_equal)
                    # weighted segment sum: magT[ch, dst] += M^T OH
                    ps_mag = psm.tile([P, P], F32, space="PSUM", tag="psmag")
                    for c in range(chunks):
                        nc.tensor.matmul(
                            out=ps_mag[:], lhsT=m_b[:, c, :], rhs=oh[:, c, :],
                            start=(c == 0), stop=(c == chunks - 1))
                    nc.vector.tensor_copy(out=mag_r[:, half * P:(half + 1) * P], in_=ps_mag[:])
                    # x tile load + PE transpose
                    xt_f = wp.tile([P, P], F32, tag="xtf")
                    nc.sync.dma_start(out=xt_f[:], in_=x_shard[t * P:(t + 1) * P, :])
                    ps_xt = pss.tile([P, P], F32, space="PSUM", tag="psxt")
                    nc.tensor.transpose(out=ps_xt[:], in_=xt_f[:], identity=id_sb[:])
                    nc.vector.tensor_copy(out=xt_r[:, half * P:(half + 1) * P], in_=ps_xt[:])

                # hT blocks: [hid128, 256] = Wl1_j^T magg + Wr1_j^T xT
                ht_r = sp.tile([P, HB, 2 * P], F32R, tag="ht")
                for j in range(HB):
                    ps_ht = psh.tile([P, 2 * P], F32, space="PSUM", tag="psht")
                    nc.tensor.matmul(out=ps_ht[:], lhsT=wl1_r[:, j * P:(j + 1) * P],
                                     rhs=mag_r[:], start=True, stop=False)
                    nc.tensor.matmul(out=ps_ht[:], lhsT=wr1_r[:, j * P:(j + 1) * P],
                                     rhs=xt_r[:], start=False, stop=True)
                    # relu(x + b1_j), round to f32r
                    nc.scalar.activation(out=ht_r[:, j, :], in_=ps_ht[:],
                                         func=AF.Relu, bias=b1_sb[:, j:j + 1], scale=1.0)
                # zrT [94, 256] = (Wl2|Wr2)^T h
                ps_zr = pss.tile([2 * OUT_CH, 2 * P], F32, space="PSUM", tag="pszr")
                for j in range(HB):
                    nc.tensor.matmul(out=ps_zr[:], lhsT=w2_r[:, j, :], rhs=ht_r[:, j, :],
                                     start=(j == 0), stop=(j == HB - 1))
                zr_f = sp.tile([2 * OUT_CH, 2 * P], F32, tag="zrf")
                nc.vector.tensor_copy(out=zr_f[:], in_=ps_zr[:])
                # transpose back per 128-node half: [94,128] -> [128,94]
                zst = sp.tile([P, 2, OUTP], F32, tag="zst")
                nc.vector.memset(zst[:], 0.0)
                rst = sp.tile([P, 2, OUTP], F32, tag="rst")
                for half in range(2):
                    ps_t = pss.tile([P, 2 * OUT_CH], F32, space="PSUM", tag="pst")
                    nc.tensor.transpose(out=ps_t[:], in_=zr_f[:, half * P:(half + 1) * P],
                                        identity=id_sb[0:2 * OUT_CH, 0:2 * OUT_CH])
                    nc.vector.tensor_copy(out=zst[:, half, 0:OUT_CH], in_=ps_t[:, 0:OUT_CH])
                    nc.vector.tensor_copy(out=rst[:, half, 0:OUT_CH],
                                          in_=ps_t[:, OUT_CH:2 * OUT_CH])
                nc.sync.dma_start(
                    out=z_out[pair * 2 * P:(pair + 1) * 2 * P, :].rearrange(
                        "(t p) c -> p t c", p=P),
                    in_=zst[:])
                nc.sync.dma_start(
                    out=r_out[pair * 2 * P:(pair + 1) * 2 * P, :].rearrange(
                        "(t p) c -> p t c", p=P),
                    in_=rst[:])
    nc.compile()
    return nc


def build_phase2(chunks: int):
    nc = bacc.Bacc("TRN2", target_bir_lowering=False, debug=False,
                   enable_asserts=False, num_devices=NCORES)
    z_all = nc.dram_tensor("z_all", [NCORES * SLOTS, OUTP], F32, kind="ExternalInput").ap()
    idx2 = nc.dram_tensor("idx2", [P, NTILES, chunks], I32, kind="ExternalInput").ap()
    dstv = nc.dram_tensor("dstv", [P, NTILES, chunks], BF16, kind="ExternalInput").ap()
    win = nc.dram_tensor("win", [P, NTILES, chunks], F32, kind="ExternalInput").ap()
    r_in = nc.dram_tensor("r_in", [SLOTS, OUTP], F32, kind="ExternalInput").ap()
    b2rep = nc.dram_tensor("b2rep", [P, OUTP], F32, kind="ExternalInput").ap()
    iota = nc.dram_tensor("iota", [P, P], BF16, kind="ExternalInput").ap()
    out = nc.dram_tensor("out", [SLOTS, OUT_CH], F32, kind="ExternalOutput").ap()

    with tile.TileContext(nc) as tc:
        with (
            tc.tile_pool(name="const", bufs=1) as cp,
            tc.tile_pool(name="work", bufs=3) as wp,
            tc.tile_pool(name="ps", bufs=4, space="PSUM") as ps,
        ):
            idx_sb = cp.tile([P, NTILES, chunks], I32)
            nc.sync.dma_start(out=idx_sb[:], in_=idx2)
            dstv_sb = cp.tile([P, NTILES, chunks], BF16)
            nc.sync.dma_start(out=dstv_sb[:], in_=dstv)
            w_sb = cp.tile([P, NTILES, chunks], F32)
            nc.sync.dma_start(out=w_sb[:], in_=win)
            iota_sb = cp.tile([P, P], BF16)
            nc.sync.dma_start(out=iota_sb[:], in_=iota)
            b2_sb = cp.tile([P, OUTP], F32)
            nc.sync.dma_start(out=b2_sb[:], in_=b2rep)
            out_stage = cp.tile([P, NTILES, OUT_CH], F32)

            for t in range(NTILES):
                m_f = wp.tile([P, chunks, OUTP], F32, tag="m")
                for c in range(chunks):
                    nc.gpsimd.indirect_dma_start(
                        out=m_f[:, c, :], out_offset=None, in_=z_all,
                        in_offset=bass.IndirectOffsetOnAxis(
                            ap=idx_sb[:, t, c:c + 1], axis=0))
                m_b = wp.tile([P, chunks, OUTP], BF16, tag="mb")
                nc.vector.tensor_tensor(
                    out=m_b[:], in0=m_f[:],
                    in1=w_sb[:, t, :].to_broadcast([P, chunks, OUTP]),
                    op=OP.mult)
                oh = wp.tile([P, chunks, P], BF16, tag="oh")
                nc.vector.tensor_tensor(
                    out=oh[:],
                    in0=dstv_sb[:, t, :].to_broadcast([P, chunks, P]),
                    in1=iota_sb[:].rearrange("p (c d) -> p c d", c=1).to_broadcast([P, chunks, P]),
                    op=OP.is_equal)
                ps_o = ps.tile([P, OUTP], F32, space="PSUM", tag="pso")
                for c in range(chunks):
                    nc.tensor.matmul(out=ps_o[:], lhsT=oh[:, c, :], rhs=m_b[:, c, :],
                                     start=(c == 0), stop=(c == chunks - 1))
                # t = agg + r + b2
                r_t = wp.tile([P, OUTP], F32, tag="rt")
                nc.sync.dma_start(out=r_t[:], in_=r_in[t * P:(t + 1) * P, :])
                t_sb = wp.tile([P, OUTP], F32, tag="tsb")
                nc.vector.tensor_tensor(out=t_sb[:], in0=ps_o[:], in1=r_t[:], op=OP.add)
                nc.vector.tensor_tensor(out=t_sb[:], in0=t_sb[:], in1=b2_sb[:], op=OP.add)
                # log_softmax over first OUT_CH cols
                rmax = wp.tile([P, 1], F32, tag="rmax")
                nc.vector.tensor_reduce(out=rmax[:], in_=t_sb[:, 0:OUT_CH],
                                        axis=mybir.AxisListType.X, op=OP.max)
                nmax = wp.tile([P, 1], F32, tag="nmax")
                nc.vector.tensor_scalar_mul(out=nmax[:], in0=rmax[:], scalar1=-1.0)
                e_sb = wp.tile([P, OUT_CH], F32, tag="esb")
                sume = wp.tile([P, 1], F32, tag="sume")
                nc.scalar.activation(out=e_sb[:], in_=t_sb[:, 0:OUT_CH], func=AF.Exp,
                                     bias=nmax[:], scale=1.0, accum_out=sume[:])
                lse = wp.tile([P, 1], F32, tag="lse")
                nc.scalar.activation(out=lse[:], in_=sume[:], func=AF.Ln)
                nc.vector.tensor_scalar(
                    out=out_stage[:, t, :], in0=t_sb[:, 0:OUT_CH],
                    scalar1=nmax[:], scalar2=lse[:], op0=OP.add, op1=OP.subtract)
            nc.sync.dma_start(
                out=out.rearrange("(t p) c -> p t c", p=P), in_=out_stage[:])
    nc.compile()
    return nc


def _prep(x, edge_index, Wl1, Wr1, b1, Wl2, Wr2, b2):
    """Host-side integer/layout preprocessing. Returns per-core input maps
    and the unpermutation for assembling the final output."""
    src = edge_index[0].astype(np.int64)
    dst = edge_index[1].astype(np.int64)
    deg = np.bincount(dst, minlength=N_NODES)
    winv = np.zeros(N_NODES, np.float32)
    winv = 1.0 / np.maximum(deg, 1).astype(np.float32)

    # per-core slot assignment: round-robin by degree over tiles
    slot_of = np.empty(N_NODES, np.int64)
    for c in range(NCORES):
        nids = np.arange(c * NPC, (c + 1) * NPC)
        order = nids[np.argsort(-deg[nids], kind="stable")]
        slots = np.empty(NPC, np.int64)
        # deal: node k -> tile k%NTILES, position k//NTILES
        k = np.arange(NPC)
        slots = (k % NTILES) * P + (k // NTILES)
        slot_of[order] = slots  # within-core slot
    core_of = np.minimum(dst // NPC, NCORES - 1)

    # group edges by (core, tile)
    dslot = slot_of[dst]
    dtile = dslot // P
    dlane = dslot % P
    ecore = dst // NPC

    max_chunk = 0
    per_core = []
    for c in range(NCORES):
        sel = np.nonzero(ecore == c)[0]
        # sort by (tile, src) for locality
        o = np.lexsort((src[sel], dtile[sel]))
        sel = sel[o]
        t_arr = dtile[sel]
        counts = np.bincount(t_arr, minlength=NTILES)
        max_chunk = max(max_chunk, int(np.ceil(counts.max() / P)))
        per_core.append((sel, counts))

    chunks = max(int(max_chunk), 1)

    idx1 = np.zeros((NCORES, P, NTILES, chunks), np.int32)
    idx2 = np.zeros((NCORES, P, NTILES, chunks), np.int32)
    dstv = np.full((NCORES, P, NTILES, chunks), -1.0, np.float32)
    wvals = np.zeros((NCORES, P, NTILES, chunks), np.float32)
    for c in range(NCORES):
        sel, counts = per_core[c]
        off = 0
        for t in range(NTILES):
            cnt = counts[t]
            es = sel[off:off + cnt]
            off += cnt
            k = np.arange(cnt)
            ch = k // P
            lane = k % P
            idx1[c, lane, t, ch] = src[es]
            sc = np.minimum(src[es] // NPC, NCORES - 1)
            idx2[c, lane, t, ch] = sc * SLOTS + slot_of[src[es]]
            dstv[c, lane, t, ch] = dlane[es]
            wvals[c, lane, t, ch] = winv[dst[es]]

    iota = np.tile(np.arange(P, dtype=np.float32)[None, :], (P, 1))
    ident = np.eye(P, dtype=np.float32)
    b1c = b1.reshape(HB, P).T.astype(np.float32).copy()  # [128, HB]
    WLR2 = np.concatenate([Wl2, Wr2], axis=1).astype(np.float32)  # [HID, 94]
    b2rep = np.zeros((P, OUTP), np.float32)
    b2rep[:, :OUT_CH] = b2[None, :]

    in1_maps, in2_maps = [], []
    xs_pad = np.zeros((NCORES, SLOTS, IN_CH), np.float32)
    for c in range(NCORES):
        nids = np.arange(c * NPC, (c + 1) * NPC)
        xs_pad[c, slot_of[nids], :] = x[nids]
        in1_maps.append({
            "x_full": x, "x_shard": xs_pad[c],
            "idx1": idx1[c], "dstv": dstv[c].astype(ml_dtypes.bfloat16),
            "win": wvals[c],
            "Wl1": Wl1, "Wr1": Wr1, "WLR2": WLR2, "b1c": b1c,
            "iota": iota.astype(ml_dtypes.bfloat16), "ident": ident,
        })
        in2_maps.append({
            "idx2": idx2[c], "dstv": dstv[c].astype(ml_dtypes.bfloat16),
            "win": wvals[c], "b2rep": b2rep,
            "iota": iota.astype(ml_dtypes.bfloat16),
        })
    return chunks, in1_maps, in2_maps, slot_of


_cache = {}


def kernel(x, edge_index, Wl1, Wr1, b1, Wl2, Wr2, b2):
    x = np.asarray(x, np.float32)
    edge_index = np.asarray(edge_index)
    chunks, in1_maps, in2_maps, slot_of = _prep(
        x, edge_index, np.asarray(Wl1, np.float32), np.asarray(Wr1, np.float32),
        np.asarray(b1, np.float32), np.asarray(Wl2, np.float32),
        np.asarray(Wr2, np.float32), np.asarray(b2, np.float32))

    if ("p1", chunks) not in _cache:
        _cache[("p1", chunks)] = build_phase1(chunks)
    nc1 = _cache[("p1", chunks)]
    res1 = bass_utils.run_bass_kernel_spmd(nc1, in1_maps, core_ids=list(range(NCORES)))
    z_all = np.concatenate([res1.results[c]["z_out"] for c in range(NCORES)], axis=0)
    for c in range(NCORES):
        in2_maps[c]["z_all"] = z_all
        in2_maps[c]["r_in"] = res1.results[c]["r_out"]

    if ("p2", chunks) not in _cache:
        _cache[("p2", chunks)] = build_phase2(chunks)
    nc2 = _cache[("p2", chunks)]
    res2 = bass_utils.run_bass_kernel_spmd(nc2, in2_maps, core_ids=list(range(NCORES)))

    out = np.empty((N_NODES, OUT_CH), np.float32)
    for c in range(NCORES):
        o = res2.results[c]["out"]  # [SLOTS, OUT_CH]
        nids = np.arange(c * NPC, (c + 1) * NPC)
        out[nids] = o[slot_of[nids]]
    return out


# ---------------------------------------------------------------------------
# timing utilities (axon has no NTFF profiling; estimate device time by
# repeated execution on persistent device buffers minus an empty baseline)
# ---------------------------------------------------------------------------

def _make_runner(nc, n_cores):
    import time
    import jax
    from jax.sharding import Mesh, PartitionSpec, NamedSharding
    from jax.experimental.shard_map import shard_map
    from concourse import bass2jax

    bass2jax.install_neuronx_cc_hook()
    pname = nc.partition_id_tensor.name if nc.partition_id_tensor else None
    in_names, out_names, out_avals = [], [], []
    for alloc in nc.m.functions[0].allocations:
        if not isinstance(alloc, mybir.MemoryLocationSet):
            continue
        name = alloc.memorylocations[0].name
        if alloc.kind == "ExternalInput":
            if name != pname:
                in_names.append(name)
        elif alloc.kind == "ExternalOutput":
            out_names.append(name)
            out_avals.append(jax.core.ShapedArray(
                tuple(alloc.tensor_shape), mybir.dt.np(alloc.dtype)))
    n_params = len(in_names)
    all_in = list(in_names) + list(out_names)
    if pname is not None:
        all_in.append(pname)

    def _body(*args):
        operands = list(args)
        if pname is not None:
            operands.append(bass2jax.partition_id_tensor())
        outs = bass2jax._bass_exec_p.bind(
            *operands, out_avals=tuple(out_avals), in_names=tuple(all_in),
            out_names=tuple(out_names), lowering_input_output_aliases=(),
            sim_require_finite=False, sim_require_nnan=False, nc=nc)
        return tuple(outs)

    devices = jax.devices()[:n_cores]
    mesh = Mesh(np.asarray(devices), ("core",))
    jitted = jax.jit(
        shard_map(_body, mesh=mesh,
                  in_specs=(PartitionSpec("core"),) * (n_params + len(out_names)),
                  out_specs=(PartitionSpec("core"),) * len(out_names),
                  check_rep=False),
        keep_unused=True)

    def prep(in_maps):
        concat = [np.concatenate([np.asarray(in_maps[c][n]) for c in range(n_cores)], 0)
                  for n in in_names]
        zeros = [np.zeros((n_cores * a.shape[0], *a.shape[1:]), a.dtype)
                 for a in out_avals]
        sh = NamedSharding(mesh, PartitionSpec("core"))
        return [jax.device_put(v, sh) for v in concat + zeros]

    def timed(dev_in, iters):
        import jax
        out = jitted(*dev_in)
        jax.block_until_ready(out)
        t0 = time.perf_counter()
        for _ in range(iters):
            out = jitted(*dev_in)
            jax.block_until_ready(out)
        return out, (time.perf_counter() - t0) / iters

    return prep, timed, out_names


def _build_empty():
    nc = bacc.Bacc("TRN2", target_bir_lowering=False, debug=False,
                   enable_asserts=False, num_devices=NCORES)
    a = nc.dram_tensor("a", [P, P], F32, kind="ExternalInput").ap()
    o = nc.dram_tensor("o", [P, P], F32, kind="ExternalOutput").ap()
    with tile.TileContext(nc) as tc:
        with tc.tile_pool(name="sb", bufs=1) as sb:
            t = sb.tile([P, P], F32)
            nc.sync.dma_start(out=t[:], in_=a)
            nc.sync.dma_start(out=o, in_=t[:])
    nc.compile()
    return nc


def measure_exec_ns(inp, iters=30):
    """Differential device-time estimate for the two kernel phases."""
    import jax
    chunks, in1_maps, in2_maps, slot_of = _prep(
        np.asarray(inp["x"], np.float32), np.asarray(inp["edge_index"]),
        np.asarray(inp["Wl1"], np.float32), np.asarray(inp["Wr1"], np.float32),
        np.asarray(inp["b1"], np.float32), np.asarray(inp["Wl2"], np.float32),
        np.asarray(inp["Wr2"], np.float32), np.asarray(inp["b2"], np.float32))
    if ("p1", chunks) not in _cache:
        _cache[("p1", chunks)] = build_phase1(chunks)
    if ("p2", chunks) not in _cache:
        _cache[("p2", chunks)] = build_phase2(chunks)

    prep0, timed0, _ = _make_runner(_build_empty(), NCORES)
    d0 = prep0([{"a": np.zeros((P, P), np.float32)} for _ in range(NCORES)])
    _, t_base = timed0(d0, iters)

    prep1, timed1, names1 = _make_runner(_cache[("p1", chunks)], NCORES)
    d1 = prep1(in1_maps)
    out1, t1 = timed1(d1, iters)
    zi = names1.index("z_out")
    ri = names1.index("r_out")
    z_all = np.asarray(out1[zi]).reshape(NCORES * SLOTS, OUTP)
    for c in range(NCORES):
        in2_maps[c]["z_all"] = z_all
        in2_maps[c]["r_in"] = np.asarray(out1[ri]).reshape(NCORES, SLOTS, OUTP)[c]

    prep2, timed2, _ = _make_runner(_cache[("p2", chunks)], NCORES)
    d2 = prep2(in2_maps)
    _, t2 = timed2(d2, iters)

    est = max(t1 - t_base, 0.0) + max(t2 - t_base, 0.0)
    print(f"  [timing] base {t_base*1e3:.2f} ms, p1 {t1*1e3:.2f} ms, "
          f"p2 {t2*1e3:.2f} ms -> est {est*1e3:.2f} ms")
    return int(est * 1e9)
